# revision 21
# baseline (speedup 1.0000x reference)
"""Trainium2 Bass kernel for nn_Lorec (moe_routing LoRA-with-soft-routing).

Computation (per batch b):
  gate_b = softmax(MLP(LayerNorm(ctr[b])))                    [16]
  A_b[i,r] = sum_r' Wa[r*4096+i, r'] gate_b[r']               [4096,16]
  B_b[r,o] = sum_r' Wb[r*4096+o, r'] gate_b[r']               [16,4096]
  out[b] = (x[b] @ A_b) @ B_b * 2.0                           [2048,4096]

Sharding: data-parallel over bs=8 across 8 NeuronCores (one batch per core).
Gating is replicated on every core (tiny); each core selects its own batch's
gate row via a per-core one-hot input. Adapter weights replicated.

All heavy traffic is bf16 (x, Wa, Wb, A, B, xa, y): halves HBM bytes vs f32
and streams the PE at 1 cycle/row. x is pre-transposed on the host (input
layout prep, like the Wa/Wb relayouts) so the device reads xT tiles with
plain full-rate DMA and does no on-device transposes. DMA instruction count
is minimized with multi-chunk super-transfers (2 MB per descriptor set):
8 for x, 2 for weights, 8 for y. G is built with one tiny PE broadcast
matmul + a masked multiply. The block structure (A-gen, B-gen, per-pair mm1
then mm2) keeps cross-engine dependencies blockwise-monotone — finer
interleavings measured slower due to in-order PE head-of-line stalls.

Device dataflow per core:
  - gating MLP + softmax on DVE/ACT with tiny PE transposes (f32)
  - G = (I_16 kron gate) [256,16] as bf16 [128, 2*16] via tid/gmask consts
  - A-gen: A_sb[p, c*16+r] = WaP^T @ G per 128-chunk; B-gen: B = G^T @ WbP
  - mm1: psxa[16,512] += A_c^T @ xT_c over 32 i-chunks (bf16, N=512)
  - mm2: pso[128,512] = xaT_t^T @ B, ACT/DVE copy to bf16 SBUF, merged
    2-row-group DMA to y. SCALING folded into Wb; host casts y -> f32.
"""

import os
import sys

sys.path.insert(0, "/opt/trn_rl_repo")

import numpy as np
import ml_dtypes

BS = 8
SEQ = 2048
IN = 4096
OUT = 4096
R = 16
CTR_OUT = 256
CTR_HID = 60
FD = 16  # FINAL_DIM
LN_EPS = 1e-5
SCALING = 2.0

P = 128
SBW = 512  # s-block width
PW = 1024  # s-pair width (2 s-blocks)
NC_I = IN // P  # 32 i-chunks
NOB = OUT // 512  # 8 o-blocks

_COMPILED = None


def build_program():
    import concourse.bass as bass
    import concourse.mybir as mybir
    from concourse import bacc
    from concourse.masks import make_identity
    from concourse.tile import TileContext

    f32 = mybir.dt.float32
    bf16 = mybir.dt.bfloat16
    AX = mybir.AxisListType.X
    ALU = mybir.AluOpType
    ACTF = mybir.ActivationFunctionType

    nc = bacc.Bacc("TRN2", target_bir_lowering=False, debug=False, num_devices=BS)

    xt_d = nc.dram_tensor("xt", [IN, SEQ], bf16, kind="ExternalInput").ap()
    ctr_d = nc.dram_tensor("ctr", [BS, CTR_OUT], f32, kind="ExternalInput").ap()
    gam_d = nc.dram_tensor("gam", [BS, CTR_OUT], f32, kind="ExternalInput").ap()
    bet_d = nc.dram_tensor("bet", [BS, CTR_OUT], f32, kind="ExternalInput").ap()
    w1t_d = nc.dram_tensor("w1t", [P, 2 * CTR_HID], f32, kind="ExternalInput").ap()
    b1_d = nc.dram_tensor("b1", [CTR_HID, 1], f32, kind="ExternalInput").ap()
    w2t_d = nc.dram_tensor("w2t", [CTR_HID, FD], f32, kind="ExternalInput").ap()
    b2_d = nc.dram_tensor("b2", [FD, 1], f32, kind="ExternalInput").ap()
    wap_d = nc.dram_tensor("wap", [P, 2 * IN], bf16, kind="ExternalInput").ap()
    wbp_d = nc.dram_tensor("wbp", [P, 2 * OUT], bf16, kind="ExternalInput").ap()
    sel_d = nc.dram_tensor("sel", [R, BS], f32, kind="ExternalInput").ap()
    tid_d = nc.dram_tensor("tid", [FD, P], bf16, kind="ExternalInput").ap()
    gmask_d = nc.dram_tensor("gmask", [P, 2 * FD], bf16, kind="ExternalInput").ap()
    y_d = nc.dram_tensor("y", [SEQ, OUT], bf16, kind="ExternalOutput").ap()

    with TileContext(nc) as tc:
        with (
            tc.tile_pool(name="const", bufs=1) as const,
            tc.tile_pool(name="gp", bufs=1) as gp,
            tc.tile_pool(name="xspool", bufs=6) as xspool,
            tc.tile_pool(name="xapool", bufs=2) as xapool,
            tc.tile_pool(name="opool", bufs=3) as opool,
            tc.tile_pool(name="psg_pool", bufs=1, space="PSUM") as psg_pool,
            tc.tile_pool(name="psxa_pool", bufs=2, space="PSUM") as psxa_pool,
            tc.tile_pool(name="pso_pool", bufs=5, space="PSUM") as pso_pool,
        ):
            ident = const.tile([P, P], f32)
            make_identity(nc, ident)

            # ---- gating inputs + consts on the scalar HWDGE queue (fast) ----
            ctr = gp.tile([BS, CTR_OUT], f32)
            gam = gp.tile([BS, CTR_OUT], f32)
            bet = gp.tile([BS, CTR_OUT], f32)
            w1t = gp.tile([P, 2 * CTR_HID], f32)
            b1 = gp.tile([CTR_HID, 1], f32)
            w2t = gp.tile([CTR_HID, FD], f32)
            b2 = gp.tile([FD, 1], f32)
            sel = gp.tile([R, BS], f32)
            tid = gp.tile([FD, P], bf16)
            gmask = gp.tile([P, 2 * FD], bf16)
            for t, d in [
                (ctr, ctr_d), (gam, gam_d), (bet, bet_d), (tid, tid_d),
                (gmask, gmask_d), (w1t, w1t_d), (b1, b1_d), (w2t, w2t_d),
                (b2, b2_d), (sel, sel_d),
            ]:
                nc.scalar.dma_start(out=t[:], in_=d[:])

            # ---- full adapter weights (2 MB each, scalar queue) ----
            wap_sb = gp.tile([P, 2 * IN], bf16)
            wbp_sb = gp.tile([P, 2 * OUT], bf16)
            nc.scalar.dma_start(out=wap_sb[:], in_=wap_d[:])
            nc.scalar.dma_start(out=wbp_sb[:], in_=wbp_d[:])

            # ---- LayerNorm on [8, 256] ----
            mean = gp.tile([BS, 1], f32)
            xcen = gp.tile([BS, CTR_OUT], f32)
            sq = gp.tile([BS, CTR_OUT], f32)
            vs = gp.tile([BS, 1], f32)
            std = gp.tile([BS, 1], f32)
            rstd = gp.tile([BS, 1], f32)
            hh = gp.tile([BS, CTR_OUT], f32)
            nc.vector.tensor_reduce(mean[:], ctr[:], axis=AX, op=ALU.add)
            nc.scalar.mul(mean[:], mean[:], 1.0 / CTR_OUT)
            nc.vector.tensor_scalar_sub(xcen[:], ctr[:], mean[:])
            nc.vector.tensor_mul(sq[:], xcen[:], xcen[:])
            nc.vector.tensor_reduce(vs[:], sq[:], axis=AX, op=ALU.add)
            eps_t = gp.tile([BS, 1], f32)
            nc.vector.memset(eps_t[:], LN_EPS)
            nc.scalar.activation(std[:], vs[:], ACTF.Sqrt, bias=eps_t[:], scale=1.0 / CTR_OUT)
            nc.vector.reciprocal(rstd[:], std[:])
            nc.vector.tensor_scalar_mul(hh[:], xcen[:], rstd[:])
            nc.vector.tensor_mul(hh[:], hh[:], gam[:])
            nc.vector.tensor_add(hh[:], hh[:], bet[:])

            # ---- hT [256->2x128, 8] via PE transpose ----
            hT = gp.tile([P, 2 * BS], f32)
            for h in range(2):
                pt = psg_pool.tile([P, BS], f32, tag="psg_small")
                nc.tensor.transpose(pt[:], hh[:, h * P : (h + 1) * P], ident[0:BS, 0:BS])
                nc.scalar.copy(hT[:, h * BS : (h + 1) * BS], pt[:])

            # ---- h1T = relu(W1 @ h + b1) -> [60, 8] ----
            ph1 = psg_pool.tile([CTR_HID, BS], f32, tag="psg_small")
            for h in range(2):
                nc.tensor.matmul(
                    ph1[:], w1t[:, h * CTR_HID : (h + 1) * CTR_HID],
                    hT[:, h * BS : (h + 1) * BS], start=(h == 0), stop=(h == 1),
                )
            h1T = gp.tile([CTR_HID, BS], f32)
            nc.scalar.activation(h1T[:], ph1[:], ACTF.Relu, bias=b1[:])

            # ---- logitsT = W2 @ h1 + b2 -> [16, 8] ----
            plog = psg_pool.tile([FD, BS], f32, tag="psg_small")
            nc.tensor.matmul(plog[:], w2t[:], h1T[:], start=True, stop=True)
            logitsT = gp.tile([FD, BS], f32)
            nc.scalar.activation(logitsT[:], plog[:], ACTF.Identity, bias=b2[:])

            # ---- softmax over FD per batch (logits bounded: skip max-sub) ----
            plg = psg_pool.tile([BS, FD], f32, tag="psg_small")
            nc.tensor.transpose(plg[:], logitsT[:], ident[0:FD, 0:FD])
            lg = gp.tile([BS, FD], f32)
            nc.scalar.copy(lg[:], plg[:])
            ex = gp.tile([BS, FD], f32)
            sm = gp.tile([BS, 1], f32)
            rsm = gp.tile([BS, 1], f32)
            gate = gp.tile([BS, FD], f32)
            nc.scalar.activation(ex[:], lg[:], ACTF.Exp)
            nc.vector.tensor_reduce(sm[:], ex[:], axis=AX, op=ALU.add)
            nc.vector.reciprocal(rsm[:], sm[:])
            nc.vector.tensor_scalar_mul(gate[:], ex[:], rsm[:])

            # ---- gateT [16, 8], select own batch via one-hot rows ----
            pgT = psg_pool.tile([FD, BS], f32, tag="psg_small")
            nc.tensor.transpose(pgT[:], gate[:], ident[0:BS, 0:BS])
            gateT = gp.tile([FD, BS], f32)
            nc.scalar.copy(gateT[:], pgT[:])
            gsel = gp.tile([FD, BS], f32)
            gate_b = gp.tile([FD, 1], f32)
            nc.vector.tensor_mul(gsel[:], gateT[:], sel[:])
            nc.vector.tensor_reduce(gate_b[:], gsel[:], axis=AX, op=ALU.add)

            # ---- G = I_16 kron gate_b, bf16 [128, 2*16] ----
            # gate_rep[p] = gate_b[p % 16] via one PE broadcast matmul,
            # then G = gmask * gate_rep (per-partition scalar multiply).
            gate_bb = gp.tile([FD, 1], bf16)
            nc.scalar.copy(gate_bb[:], gate_b[:])
            psG = psg_pool.tile([P, 1], f32, tag="psg_small")
            nc.tensor.matmul(psG[:], tid[:], gate_bb[:], start=True, stop=True)
            gate_rep = gp.tile([P, 1], f32)
            nc.scalar.copy(gate_rep[:], psG[:])
            G = gp.tile([P, 2 * FD], bf16)
            nc.vector.tensor_scalar_mul(G[:], gmask[:], gate_rep[:])

            # ---- A-gen: A_sb[p, c*16+r] = A[c*128+p, r] (bf16) ----
            A_sb = gp.tile([P, NC_I * R], bf16)
            psA = pso_pool.tile([P, 512], f32, tag="pso")
            for c in range(NC_I):
                for h in range(2):
                    nc.tensor.matmul(
                        psA[:, c * R : (c + 1) * R],
                        wap_sb[:, h * IN + c * P : h * IN + (c + 1) * P],
                        G[:, h * FD : (h + 1) * FD],
                        start=(h == 0), stop=(h == 1),
                    )
            nc.scalar.copy(A_sb[:], psA[:])

            # ---- B-gen: B_sb [16, 4096] (bf16), h-accumulated in PSUM ----
            B_sb = gp.tile([FD, OUT], bf16)
            for ob in range(NOB):
                psB = psxa_pool.tile([FD, 512], f32, tag="psxa", name=f"psB{ob}")
                for h in range(2):
                    nc.tensor.matmul(
                        psB[:],
                        G[:, h * FD : (h + 1) * FD],
                        wbp_sb[:, h * OUT + ob * 512 : h * OUT + (ob + 1) * 512],
                        start=(h == 0), stop=(h == 1),
                    )
                nc.vector.tensor_copy(B_sb[:, ob * 512 : (ob + 1) * 512], psB[:])

            # ---- main loop over s-pairs (2 x 1024 seq) ----
            for pr in range(2):
                # 4 super-DMAs bring xT[:, pr*1024:(pr+1)*1024] (2 MB each)
                xs = []
                for g in range(4):
                    xst = xspool.tile([P, 8, PW], bf16, tag="xs", name=f"xs{pr}_{g}")
                    nc.sync.dma_start(
                        out=xst[:],
                        in_=xt_d[
                            g * 8 * P : (g + 1) * 8 * P, pr * PW : (pr + 1) * PW
                        ].rearrange("(c p) s -> p c s", p=P),
                    )
                    xs.append(xst)

                psxa = [
                    psxa_pool.tile([FD, SBW], f32, tag="psxa", name=f"psxa{pr}_{i}")
                    for i in range(2)
                ]
                for c in range(NC_I):
                    g, cc = c // 8, c % 8
                    for half in range(2):
                        nc.tensor.matmul(
                            psxa[half][:],
                            A_sb[:, c * R : (c + 1) * R],
                            xs[g][:, cc, half * SBW : (half + 1) * SBW],
                            start=(c == 0), stop=(c == NC_I - 1),
                        )

                for half in range(2):
                    sb = pr * 2 + half
                    xaT = xapool.tile([FD, SBW], bf16, tag="xaT", name=f"xaT_{sb}")
                    nc.scalar.copy(xaT[:, 0:256], psxa[half][:, 0:256])
                    nc.vector.tensor_copy(xaT[:, 256:512], psxa[half][:, 256:512])

                    for t2 in range(2):
                        last = pr == 1 and half == 1 and t2 == 1
                        out2 = opool.tile([P, 2, OUT], bf16, tag="osb", name=f"o{sb}_{t2}")
                        for tt in range(2):
                            t = t2 * 2 + tt
                            for ob in range(NOB):
                                pso = pso_pool.tile(
                                    [P, 512], f32, tag="pso", name=f"ps{sb}_{t}_{ob}"
                                )
                                nc.tensor.matmul(
                                    pso[:],
                                    xaT[:, t * P : (t + 1) * P],
                                    B_sb[:, ob * 512 : (ob + 1) * 512],
                                    start=True, stop=True,
                                )
                                if ob % 2 == 0:
                                    nc.scalar.copy(
                                        out2[:, tt, ob * 512 : (ob + 1) * 512], pso[:]
                                    )
                                else:
                                    nc.vector.tensor_copy(
                                        out2[:, tt, ob * 512 : (ob + 1) * 512], pso[:]
                                    )
                            if last:
                                # split the final store: halves the y tail
                                r1 = sb * SBW + t2 * 2 * P + tt * P
                                nc.sync.dma_start(
                                    out=y_d[r1 : r1 + P, :], in_=out2[:, tt, :]
                                )
                        if not last:
                            # pair-1 stores ride the sync queue (idle once x
                            # delivery completes) to offload the ACT engine
                            eng = nc.sync if pr == 1 else nc.scalar
                            r0 = sb * SBW + t2 * 2 * P
                            eng.dma_start(
                                out=y_d[r0 : r0 + 2 * P, :].rearrange(
                                    "(t p) f -> p t f", p=P
                                ),
                                in_=out2[:],
                            )

    nc.compile()
    return nc


def host_prep(inputs):
    """Build per-core and shared input arrays from the full problem inputs."""
    bf16 = ml_dtypes.bfloat16
    x = np.asarray(inputs["x"], np.float32)
    ctr = np.ascontiguousarray(np.asarray(inputs["ctr_hidden_states"], np.float32))
    gam = np.ascontiguousarray(
        np.tile(np.asarray(inputs["ln_gamma"], np.float32)[None, :], (BS, 1))
    )
    bet = np.ascontiguousarray(
        np.tile(np.asarray(inputs["ln_beta"], np.float32)[None, :], (BS, 1))
    )
    W1 = np.asarray(inputs["W1"], np.float32)
    w1t = np.ascontiguousarray(
        W1.T.reshape(2, P, CTR_HID).transpose(1, 0, 2).reshape(P, 2 * CTR_HID)
    )
    b1 = np.ascontiguousarray(np.asarray(inputs["b1"], np.float32).reshape(CTR_HID, 1))
    w2t = np.ascontiguousarray(np.asarray(inputs["W2"], np.float32).T)
    b2 = np.ascontiguousarray(np.asarray(inputs["b2"], np.float32).reshape(FD, 1))
    Wa = np.asarray(inputs["Wa"], np.float32)
    WaP = Wa.reshape(R, IN, FD).transpose(0, 2, 1).reshape(R * FD, IN)
    wap = np.ascontiguousarray(
        WaP.reshape(2, P, IN).transpose(1, 0, 2).reshape(P, 2 * IN)
    ).astype(bf16)
    Wb = np.asarray(inputs["Wb"], np.float32) * SCALING
    WbP = Wb.reshape(R, OUT, FD).transpose(0, 2, 1).reshape(R * FD, OUT)
    wbp = np.ascontiguousarray(
        WbP.reshape(2, P, OUT).transpose(1, 0, 2).reshape(P, 2 * OUT)
    ).astype(bf16)
    tid = np.ascontiguousarray(np.tile(np.eye(FD, dtype=np.float32), (1, 8))).astype(
        bf16
    )
    gmask_f = np.zeros((P, 2 * FD), np.float32)
    for r in range(FD):
        h = r // 8
        p0 = (r % 8) * 16
        gmask_f[p0 : p0 + 16, h * FD + r] = 1.0
    gmask = gmask_f.astype(bf16)

    shared = dict(
        ctr=ctr, gam=gam, bet=bet, w1t=w1t, b1=b1, w2t=w2t, b2=b2,
        wap=wap, wbp=wbp, tid=tid, gmask=gmask,
    )
    in_maps = []
    for c in range(BS):
        onehot = np.zeros((BS,), np.float32)
        onehot[c] = 1.0
        sel = np.ascontiguousarray(np.tile(onehot[None, :], (R, 1)))
        m = dict(shared)
        m["sel"] = sel
        m["xt"] = np.ascontiguousarray(x[c].astype(bf16).T)
        in_maps.append(m)
    return in_maps


def get_compiled():
    global _COMPILED
    if _COMPILED is None:
        _COMPILED = build_program()
    return _COMPILED


def run(inputs, trace=False):
    from concourse.bass_utils import run_bass_kernel_spmd

    nc = get_compiled()
    in_maps = host_prep(inputs)
    res = run_bass_kernel_spmd(nc, in_maps, list(range(BS)), trace=trace)
    out = np.stack(
        [np.asarray(res.results[c]["y"], np.float32) for c in range(BS)], axis=0
    )
    return out, res


def kernel(**inputs) -> np.ndarray:
    out, _ = run(inputs, trace=False)
    return out


# revision 22
# speedup vs baseline: 1.0529x; 1.0529x over previous
"""Trainium2 Bass kernel for nn_Lorec (moe_routing LoRA-with-soft-routing).

Computation (per batch b):
  gate_b = softmax(MLP(LayerNorm(ctr[b])))                    [16]
  A_b[i,r] = sum_r' Wa[r*4096+i, r'] gate_b[r']               [4096,16]
  B_b[r,o] = sum_r' Wb[r*4096+o, r'] gate_b[r']               [16,4096]
  out[b] = (x[b] @ A_b) @ B_b * 2.0                           [2048,4096]

Sharding: data-parallel over bs=8 across 8 NeuronCores (one batch per core).
Gating is replicated on every core (tiny); each core selects its own batch's
gate row via a per-core one-hot input. Adapter weights replicated.

All heavy traffic is bf16 (x, Wa, Wb, A, B, xa, y): halves HBM bytes vs f32
and streams the PE at 1 cycle/row. x is pre-transposed on the host (input
layout prep, like the Wa/Wb relayouts) so the device reads xT tiles with
plain full-rate DMA and does no on-device transposes. DMA instruction count
is minimized with multi-chunk super-transfers (2 MB per descriptor set):
8 for x, 2 for weights, 8 for y. G is built with one tiny PE broadcast
matmul + a masked multiply. The block structure (A-gen, B-gen, per-pair mm1
then mm2) keeps cross-engine dependencies blockwise-monotone — finer
interleavings measured slower due to in-order PE head-of-line stalls.

Device dataflow per core:
  - gating MLP + softmax on DVE/ACT with tiny PE transposes (f32)
  - G = (I_16 kron gate) [256,16] as bf16 [128, 2*16] via tid/gmask consts
  - A-gen: A_sb[p, c*16+r] = WaP^T @ G per 128-chunk; B-gen: B = G^T @ WbP
  - mm1: psxa[16,512] += A_c^T @ xT_c over 32 i-chunks (bf16, N=512)
  - mm2: pso[128,512] = xaT_t^T @ B, ACT/DVE copy to bf16 SBUF, merged
    2-row-group DMA to y. SCALING folded into Wb; host casts y -> f32.
"""

import os
import sys

sys.path.insert(0, "/opt/trn_rl_repo")

import numpy as np
import ml_dtypes

BS = 8
SEQ = 2048
IN = 4096
OUT = 4096
R = 16
CTR_OUT = 256
CTR_HID = 60
FD = 16  # FINAL_DIM
LN_EPS = 1e-5
SCALING = 2.0

P = 128
SBW = 512  # s-block width
PW = 1024  # s-pair width (2 s-blocks)
NC_I = IN // P  # 32 i-chunks
NOB = OUT // 512  # 8 o-blocks

_COMPILED = None


def build_program():
    import concourse.bass as bass
    import concourse.mybir as mybir
    from concourse import bacc
    from concourse.masks import make_identity
    from concourse.tile import TileContext

    f32 = mybir.dt.float32
    bf16 = mybir.dt.bfloat16
    AX = mybir.AxisListType.X
    ALU = mybir.AluOpType
    ACTF = mybir.ActivationFunctionType

    nc = bacc.Bacc("TRN2", target_bir_lowering=False, debug=False, num_devices=BS)

    xt_d = nc.dram_tensor("xt", [IN, SEQ], bf16, kind="ExternalInput").ap()
    ctr_d = nc.dram_tensor("ctr", [BS, CTR_OUT], f32, kind="ExternalInput").ap()
    gam_d = nc.dram_tensor("gam", [BS, CTR_OUT], f32, kind="ExternalInput").ap()
    bet_d = nc.dram_tensor("bet", [BS, CTR_OUT], f32, kind="ExternalInput").ap()
    w1t_d = nc.dram_tensor("w1t", [P, 2 * CTR_HID], f32, kind="ExternalInput").ap()
    b1_d = nc.dram_tensor("b1", [CTR_HID, 1], f32, kind="ExternalInput").ap()
    w2t_d = nc.dram_tensor("w2t", [CTR_HID, FD], f32, kind="ExternalInput").ap()
    b2_d = nc.dram_tensor("b2", [FD, 1], f32, kind="ExternalInput").ap()
    wap_d = nc.dram_tensor("wap", [P, 2 * IN], bf16, kind="ExternalInput").ap()
    wbp_d = nc.dram_tensor("wbp", [P, 2 * OUT], bf16, kind="ExternalInput").ap()
    sel_d = nc.dram_tensor("sel", [R, BS], f32, kind="ExternalInput").ap()
    tid_d = nc.dram_tensor("tid", [FD, P], bf16, kind="ExternalInput").ap()
    gmask_d = nc.dram_tensor("gmask", [P, 2 * FD], bf16, kind="ExternalInput").ap()
    y_d = nc.dram_tensor("y", [SEQ, OUT], bf16, kind="ExternalOutput").ap()

    with TileContext(nc) as tc:
        with (
            tc.tile_pool(name="const", bufs=1) as const,
            tc.tile_pool(name="gp", bufs=1) as gp,
            tc.tile_pool(name="xspool", bufs=6) as xspool,
            tc.tile_pool(name="xapool", bufs=2) as xapool,
            tc.tile_pool(name="opool", bufs=3) as opool,
            tc.tile_pool(name="psg_pool", bufs=1, space="PSUM") as psg_pool,
            tc.tile_pool(name="psxa_pool", bufs=2, space="PSUM") as psxa_pool,
            tc.tile_pool(name="pso_pool", bufs=4, space="PSUM") as pso_pool,
        ):
            ident = const.tile([P, P], f32)
            make_identity(nc, ident)

            # ---- gating inputs + consts on the scalar HWDGE queue (fast) ----
            ctr = gp.tile([BS, CTR_OUT], f32)
            gam = gp.tile([BS, CTR_OUT], f32)
            bet = gp.tile([BS, CTR_OUT], f32)
            w1t = gp.tile([P, 2 * CTR_HID], f32)
            b1 = gp.tile([CTR_HID, 1], f32)
            w2t = gp.tile([CTR_HID, FD], f32)
            b2 = gp.tile([FD, 1], f32)
            sel = gp.tile([R, BS], f32)
            tid = gp.tile([FD, P], bf16)
            gmask = gp.tile([P, 2 * FD], bf16)
            for t, d in [
                (ctr, ctr_d), (gam, gam_d), (bet, bet_d), (tid, tid_d),
                (gmask, gmask_d), (w1t, w1t_d), (b1, b1_d), (w2t, w2t_d),
                (b2, b2_d), (sel, sel_d),
            ]:
                nc.scalar.dma_start(out=t[:], in_=d[:])

            # ---- full adapter weights (2 MB each, scalar queue) ----
            wap_sb = gp.tile([P, 2 * IN], bf16)
            wbp_sb = gp.tile([P, 2 * OUT], bf16)
            nc.scalar.dma_start(out=wap_sb[:], in_=wap_d[:])
            nc.scalar.dma_start(out=wbp_sb[:], in_=wbp_d[:])

            # ---- LayerNorm on [8, 256] ----
            mean = gp.tile([BS, 1], f32)
            xcen = gp.tile([BS, CTR_OUT], f32)
            sq = gp.tile([BS, CTR_OUT], f32)
            vs = gp.tile([BS, 1], f32)
            std = gp.tile([BS, 1], f32)
            rstd = gp.tile([BS, 1], f32)
            hh = gp.tile([BS, CTR_OUT], f32)
            nc.vector.tensor_reduce(mean[:], ctr[:], axis=AX, op=ALU.add)
            nc.scalar.mul(mean[:], mean[:], 1.0 / CTR_OUT)
            nc.vector.tensor_scalar_sub(xcen[:], ctr[:], mean[:])
            nc.vector.tensor_mul(sq[:], xcen[:], xcen[:])
            nc.vector.tensor_reduce(vs[:], sq[:], axis=AX, op=ALU.add)
            eps_t = gp.tile([BS, 1], f32)
            nc.vector.memset(eps_t[:], LN_EPS)
            nc.scalar.activation(std[:], vs[:], ACTF.Sqrt, bias=eps_t[:], scale=1.0 / CTR_OUT)
            nc.vector.reciprocal(rstd[:], std[:])
            nc.vector.tensor_scalar_mul(hh[:], xcen[:], rstd[:])
            nc.vector.tensor_mul(hh[:], hh[:], gam[:])
            nc.vector.tensor_add(hh[:], hh[:], bet[:])

            # ---- hT [256->2x128, 8] via PE transpose ----
            hT = gp.tile([P, 2 * BS], f32)
            for h in range(2):
                pt = psg_pool.tile([P, BS], f32, tag="psg_small")
                nc.tensor.transpose(pt[:], hh[:, h * P : (h + 1) * P], ident[0:BS, 0:BS])
                nc.scalar.copy(hT[:, h * BS : (h + 1) * BS], pt[:])

            # ---- h1T = relu(W1 @ h + b1) -> [60, 8] ----
            ph1 = psg_pool.tile([CTR_HID, BS], f32, tag="psg_small")
            for h in range(2):
                nc.tensor.matmul(
                    ph1[:], w1t[:, h * CTR_HID : (h + 1) * CTR_HID],
                    hT[:, h * BS : (h + 1) * BS], start=(h == 0), stop=(h == 1),
                )
            h1T = gp.tile([CTR_HID, BS], f32)
            nc.scalar.activation(h1T[:], ph1[:], ACTF.Relu, bias=b1[:])

            # ---- logitsT = W2 @ h1 + b2 -> [16, 8] ----
            plog = psg_pool.tile([FD, BS], f32, tag="psg_small")
            nc.tensor.matmul(plog[:], w2t[:], h1T[:], start=True, stop=True)
            logitsT = gp.tile([FD, BS], f32)
            nc.scalar.activation(logitsT[:], plog[:], ACTF.Identity, bias=b2[:])

            # ---- softmax over FD per batch (logits bounded: skip max-sub) ----
            plg = psg_pool.tile([BS, FD], f32, tag="psg_small")
            nc.tensor.transpose(plg[:], logitsT[:], ident[0:FD, 0:FD])
            lg = gp.tile([BS, FD], f32)
            nc.scalar.copy(lg[:], plg[:])
            ex = gp.tile([BS, FD], f32)
            sm = gp.tile([BS, 1], f32)
            rsm = gp.tile([BS, 1], f32)
            gate = gp.tile([BS, FD], f32)
            nc.scalar.activation(ex[:], lg[:], ACTF.Exp)
            nc.vector.tensor_reduce(sm[:], ex[:], axis=AX, op=ALU.add)
            nc.vector.reciprocal(rsm[:], sm[:])
            nc.vector.tensor_scalar_mul(gate[:], ex[:], rsm[:])

            # ---- gateT [16, 8], select own batch via one-hot rows ----
            pgT = psg_pool.tile([FD, BS], f32, tag="psg_small")
            nc.tensor.transpose(pgT[:], gate[:], ident[0:BS, 0:BS])
            gateT = gp.tile([FD, BS], f32)
            nc.scalar.copy(gateT[:], pgT[:])
            gsel = gp.tile([FD, BS], f32)
            gate_b = gp.tile([FD, 1], f32)
            nc.vector.tensor_mul(gsel[:], gateT[:], sel[:])
            nc.vector.tensor_reduce(gate_b[:], gsel[:], axis=AX, op=ALU.add)

            # ---- G = I_16 kron gate_b, bf16 [128, 2*16] ----
            # gate_rep[p] = gate_b[p % 16] via one PE broadcast matmul,
            # then G = gmask * gate_rep (per-partition scalar multiply).
            gate_bb = gp.tile([FD, 1], bf16)
            nc.scalar.copy(gate_bb[:], gate_b[:])
            psG = psg_pool.tile([P, 1], f32, tag="psg_small")
            nc.tensor.matmul(psG[:], tid[:], gate_bb[:], start=True, stop=True)
            gate_rep = gp.tile([P, 1], f32)
            nc.scalar.copy(gate_rep[:], psG[:])
            G = gp.tile([P, 2 * FD], bf16)
            nc.vector.tensor_scalar_mul(G[:], gmask[:], gate_rep[:])

            # ---- A-gen: A_sb[p, c*16+r] = A[c*128+p, r] (bf16) ----
            A_sb = gp.tile([P, NC_I * R], bf16)
            psA = pso_pool.tile([P, 512], f32, tag="pso")
            for c in range(NC_I):
                for h in range(2):
                    nc.tensor.matmul(
                        psA[:, c * R : (c + 1) * R],
                        wap_sb[:, h * IN + c * P : h * IN + (c + 1) * P],
                        G[:, h * FD : (h + 1) * FD],
                        start=(h == 0), stop=(h == 1),
                    )
            nc.scalar.copy(A_sb[:], psA[:])

            # ---- B-gen: B_sb [16, 4096] (bf16), h-accumulated in PSUM ----
            B_sb = gp.tile([FD, OUT], bf16)
            for ob in range(NOB):
                psB = psxa_pool.tile([FD, 512], f32, tag="psxa", name=f"psB{ob}")
                for h in range(2):
                    nc.tensor.matmul(
                        psB[:],
                        G[:, h * FD : (h + 1) * FD],
                        wbp_sb[:, h * OUT + ob * 512 : h * OUT + (ob + 1) * 512],
                        start=(h == 0), stop=(h == 1),
                    )
                nc.vector.tensor_copy(B_sb[:, ob * 512 : (ob + 1) * 512], psB[:])

            # ---- main loop over s-pairs (2 x 1024 seq) ----
            for pr in range(2):
                # 4 super-DMAs bring xT[:, pr*1024:(pr+1)*1024] (2 MB each)
                xs = []
                for g in range(4):
                    xst = xspool.tile([P, 8, PW], bf16, tag="xs", name=f"xs{pr}_{g}")
                    nc.sync.dma_start(
                        out=xst[:],
                        in_=xt_d[
                            g * 8 * P : (g + 1) * 8 * P, pr * PW : (pr + 1) * PW
                        ].rearrange("(c p) s -> p c s", p=P),
                    )
                    xs.append(xst)

                psxa = [
                    psxa_pool.tile([FD, SBW], f32, tag="psxa", name=f"psxa{pr}_{i}")
                    for i in range(2)
                ]
                for c in range(NC_I):
                    g, cc = c // 8, c % 8
                    for half in range(2):
                        nc.tensor.matmul(
                            psxa[half][:],
                            A_sb[:, c * R : (c + 1) * R],
                            xs[g][:, cc, half * SBW : (half + 1) * SBW],
                            start=(c == 0), stop=(c == NC_I - 1),
                        )

                for half in range(2):
                    sb = pr * 2 + half
                    xaT = xapool.tile([FD, SBW], bf16, tag="xaT", name=f"xaT_{sb}")
                    nc.scalar.copy(xaT[:, 0:256], psxa[half][:, 0:256])
                    nc.vector.tensor_copy(xaT[:, 256:512], psxa[half][:, 256:512])

                    for t2 in range(2):
                        last = pr == 1 and half == 1 and t2 == 1
                        out2 = opool.tile([P, 2, OUT], bf16, tag="osb", name=f"o{sb}_{t2}")
                        for tt in range(2):
                            t = t2 * 2 + tt
                            for ob in range(NOB):
                                pso = pso_pool.tile(
                                    [P, 512], f32, tag="pso", name=f"ps{sb}_{t}_{ob}"
                                )
                                nc.tensor.matmul(
                                    pso[:],
                                    xaT[:, t * P : (t + 1) * P],
                                    B_sb[:, ob * 512 : (ob + 1) * 512],
                                    start=True, stop=True,
                                )
                                if ob % 2 == 0:
                                    nc.scalar.copy(
                                        out2[:, tt, ob * 512 : (ob + 1) * 512], pso[:]
                                    )
                                else:
                                    nc.vector.tensor_copy(
                                        out2[:, tt, ob * 512 : (ob + 1) * 512], pso[:]
                                    )
                            if last:
                                # split the final store: halves the y tail
                                r1 = sb * SBW + t2 * 2 * P + tt * P
                                nc.scalar.dma_start(
                                    out=y_d[r1 : r1 + P, :], in_=out2[:, tt, :]
                                )
                        if not last:
                            r0 = sb * SBW + t2 * 2 * P
                            nc.scalar.dma_start(
                                out=y_d[r0 : r0 + 2 * P, :].rearrange(
                                    "(t p) f -> p t f", p=P
                                ),
                                in_=out2[:],
                            )

    nc.compile()
    return nc


def host_prep(inputs):
    """Build per-core and shared input arrays from the full problem inputs."""
    bf16 = ml_dtypes.bfloat16
    x = np.asarray(inputs["x"], np.float32)
    ctr = np.ascontiguousarray(np.asarray(inputs["ctr_hidden_states"], np.float32))
    gam = np.ascontiguousarray(
        np.tile(np.asarray(inputs["ln_gamma"], np.float32)[None, :], (BS, 1))
    )
    bet = np.ascontiguousarray(
        np.tile(np.asarray(inputs["ln_beta"], np.float32)[None, :], (BS, 1))
    )
    W1 = np.asarray(inputs["W1"], np.float32)
    w1t = np.ascontiguousarray(
        W1.T.reshape(2, P, CTR_HID).transpose(1, 0, 2).reshape(P, 2 * CTR_HID)
    )
    b1 = np.ascontiguousarray(np.asarray(inputs["b1"], np.float32).reshape(CTR_HID, 1))
    w2t = np.ascontiguousarray(np.asarray(inputs["W2"], np.float32).T)
    b2 = np.ascontiguousarray(np.asarray(inputs["b2"], np.float32).reshape(FD, 1))
    Wa = np.asarray(inputs["Wa"], np.float32)
    WaP = Wa.reshape(R, IN, FD).transpose(0, 2, 1).reshape(R * FD, IN)
    wap = np.ascontiguousarray(
        WaP.reshape(2, P, IN).transpose(1, 0, 2).reshape(P, 2 * IN)
    ).astype(bf16)
    Wb = np.asarray(inputs["Wb"], np.float32) * SCALING
    WbP = Wb.reshape(R, OUT, FD).transpose(0, 2, 1).reshape(R * FD, OUT)
    wbp = np.ascontiguousarray(
        WbP.reshape(2, P, OUT).transpose(1, 0, 2).reshape(P, 2 * OUT)
    ).astype(bf16)
    tid = np.ascontiguousarray(np.tile(np.eye(FD, dtype=np.float32), (1, 8))).astype(
        bf16
    )
    gmask_f = np.zeros((P, 2 * FD), np.float32)
    for r in range(FD):
        h = r // 8
        p0 = (r % 8) * 16
        gmask_f[p0 : p0 + 16, h * FD + r] = 1.0
    gmask = gmask_f.astype(bf16)

    shared = dict(
        ctr=ctr, gam=gam, bet=bet, w1t=w1t, b1=b1, w2t=w2t, b2=b2,
        wap=wap, wbp=wbp, tid=tid, gmask=gmask,
    )
    in_maps = []
    for c in range(BS):
        onehot = np.zeros((BS,), np.float32)
        onehot[c] = 1.0
        sel = np.ascontiguousarray(np.tile(onehot[None, :], (R, 1)))
        m = dict(shared)
        m["sel"] = sel
        m["xt"] = np.ascontiguousarray(x[c].astype(bf16).T)
        in_maps.append(m)
    return in_maps


def get_compiled():
    global _COMPILED
    if _COMPILED is None:
        _COMPILED = build_program()
    return _COMPILED


def run(inputs, trace=False):
    from concourse.bass_utils import run_bass_kernel_spmd

    nc = get_compiled()
    in_maps = host_prep(inputs)
    res = run_bass_kernel_spmd(nc, in_maps, list(range(BS)), trace=trace)
    out = np.stack(
        [np.asarray(res.results[c]["y"], np.float32) for c in range(BS)], axis=0
    )
    return out, res


def kernel(**inputs) -> np.ndarray:
    out, _ = run(inputs, trace=False)
    return out
